# revision 25
# baseline (speedup 1.0000x reference)
"""Trainium2 Bass kernel for nn_BSLoss (Black-Scholes PINN loss on a 4096x4096 grid).

Strategy (8 NeuronCores, SPMD, S-sharded, bf16 transfers; ~42us vs 50us f32
baseline):
  - Host casts V to bf16: HBM traffic halves to ~4.2MB/core. The loss is a
    16.7M-term squared sum, so elementwise rounding washes out (rel err ~4e-4
    vs the 2e-2 budget).
  - Each core: 512 rows (+1-row halos) x 4096 cols as 4 x 128-row tiles
    (step 126) + a 10-row strip. Per full tile, two 2048-wide column groups
    ([128,2048] f32 PSUM tile = 4 banks; 2 in flight = all 8 banks).
  - residual/C_T = tri_S(V) + (V[:,j+1] - V[:,j-1]):
      * tri_S: [128x128] bf16 stationary matmuls (512-col chunks - the ISA
        caps matmul output at one PSUM bank; wider output fails codegen).
      * t-shift: DVE tensor_tensor sub in bf16 (2x mode) into a w tile,
        then added into PSUM by identity-stationary matmuls on PE.
        (DVE STT adds into PSUM were tried and serialize the pipeline:
        sub -> STT -> square chains plus PSUM buffer reuse stall PE.)
      * square+row-accumulate: ScalarE activation(Square, accum_out) per
        group; two LATE groups spilled to DVE copy+STT (an instruction may
        read only ONE operand from PSUM, hence the copy) to shorten the
        Act tail without blocking DVE's in-order sub queue.
  - The strip is host-packed into [120, 344]: 12 column-blocks of 342 out
    cols stacked along partitions, with block-diagonal tri/identity
    stationaries - full-width strip ops would cost like a 5th tile. It is
    computed FIRST to fill the pipeline ramp-up; the junk edge columns
    (global col 0 + clip-padded tail) are recomputed and subtracted host-side.
  - v tiles stream as whole [128,4096] DMAs on the Sync HWDGE queue in
    compute order (fans out over all 16 DMA engines, ~400GB/s); weights ride
    first on Sync; stats return split across the Scalar/Sync queues.
  - Host applies row masks (x C_T^2, the folded-out time-step scale) to the
    [128,10] per-row stats, sums in float64, and computes the O(N) boundary
    losses (rows 0/4095, col 4095) directly.

Measured notes (NTFF profiles): engines run well below nominal clocks here
(PE ~1.2GHz vs 2.4 spec, Act ~0.87GHz vs 1.2; throttle_avg_util_limit ~0.5,
worse while DMA streams), and ~11us prologue + ~4us tail are fixed runtime
costs, which bounds this structure at roughly 40us.
"""
import os
import sys

if "/opt/trn_rl_repo" not in sys.path:
    sys.path.insert(0, "/opt/trn_rl_repo")

import numpy as np
import ml_dtypes

import concourse.mybir as mybir
import concourse.tile as tile
from concourse import bacc
from concourse.bass_utils import run_bass_kernel_spmd

BF16NP = ml_dtypes.bfloat16

# ---- problem constants (match the reference) ----
N_S, N_T = 4096, 4096
R, SIGMA, K, T_MAT, SMAX = 0.05, 0.2, 100.0, 1.0, 300.0
B_STR, ALPHA = K / SMAX, 0.5
L_PDE, L_BC, L_TC = 1.0, 10.0, 10.0
HUBER_DELTA = 0.01
SOFTPLUS_BETA = 50.0

N_CORES = 8
ROWS_PER_CORE = N_S // N_CORES          # 512
IN_ROWS = ROWS_PER_CORE + 2             # 514 (with halos)
P = 128
TILE_STARTS = [0, 126, 252, 378]        # full tiles; outputs local rows 1..504
STRIP_START = 504                       # strip rows 504..513 -> outputs 505..512
STRIP_K = 10
N_FULL = 4
C_T = (N_T - 1) / 2.0 / T_MAT           # 2047.5

# column groups of full tiles (global col base, width); interior cols 1..4094
G_BASE = [1, 2049]
G_W = [2048, 2046]
# strip pack: 12 column-blocks of 342 out-cols stacked along partitions
SQ_N = 12                                # blocks
SQ_BW = 342                              # out cols per block (12*342 >= 4094)
SQ_W = SQ_BW + 2                         # + 2 halo cols
N_GROUPS = 10                            # 0..7 full-tile, 8 strip, 9 probe
U_STRIP = 8
U_PROBE = 9
# full-tile groups whose t-shift add runs on DVE (STT into PSUM).
# Empty: the sub->STT->square chain serializes the pipeline (measured v3);
# identity matmuls on PE drain PSUM groups much earlier.
ADD_DVE = set()
SQ_DVE = {(2, 1)}                        # (tile, g) squares spilled to DVE.
                                         # Late groups only: their copy+STT
                                         # sit at the END of DVE's in-order
                                         # queue, after all sub TTs — earlier
                                         # placements block later tiles' subs
                                         # and serialize the pipeline
                                         # (measured: {(1,1),(2,0)} -> 52us).
                                         # One spill only: with two, the
                                         # second's copy+STT became the tail
                                         # while Act sat idle (v5 trace).

F32 = mybir.dt.float32
BF16 = mybir.dt.bfloat16


def _solve_cubic(Q: float) -> float:
    c = -Q
    for _ in range(5):
        f = c ** 3 / 6.0 + c + Q
        df = 0.5 * c * c + 1.0
        c = c - f / df
    return c


C1 = _solve_cubic((B_STR - 0.0) / ALPHA)
C2 = _solve_cubic((B_STR - 1.0) / ALPHA)


def _stencil_coeffs(S: np.ndarray):
    """Per-row stencil coefficients / C_T (C_T folded out; re-applied via host mask)."""
    S = S.astype(np.float64)
    dS = 1.0 / (N_S - 1)
    L = C2 * S + C1 * (1.0 - S)
    dL = C2 - C1
    S_u = ALPHA * dL * (0.5 * L ** 2 + 1.0)
    S_uu = ALPHA * dL ** 2 * L
    e = 0.5 * SIGMA ** 2 * S ** 2
    f = R * S
    a_uu = e / S_u ** 2
    a_u = f / S_u - e * S_uu / S_u ** 3
    hi = a_uu / dS ** 2 + a_u / (2 * dS)
    lo = a_uu / dS ** 2 - a_u / (2 * dS)
    mid = -2.0 * a_uu / dS ** 2 - R
    return lo / C_T, mid / C_T, hi / C_T


_PROGRAM = None


def _patch_tail(tc_cls):
    """Cheaper kernel tail: drain + single barrier, no per-sem HW clears."""
    from concourse.vector_clock import ScopedClock as _SC

    def _drain_and_barrier(self, tick_clock, wait_clock):
        drain_inst = self.nc.sync.drain()
        wait_clock.add_sem_waits(drain_inst.ins, _SC({None: tick_clock.global_clock}))
        self.nc.all_engine_barrier()
        popped = self.nc._tile_sem_poison_stack.pop()
        assert popped is self._sem_poison
        sems = list(self.sems.allocated().values())
        sem_nums = [s.num if hasattr(s, "num") else s for s in sems]
        self.nc._state.prepend_free_semaphores(sem_nums)
        for poison_set in self.nc._tile_sem_poison_stack:
            poison_set.update(sem_nums)

    tc_cls._drain_and_barrier = _drain_and_barrier


def _build_program():
    if os.environ.get("BSLOSS_FAST_TAIL", "1") == "1":
        _patch_tail(tile.TileContext)
    nc = bacc.Bacc("TRN2", target_bir_lowering=False)

    v_in = nc.dram_tensor("v_in", [IN_ROWS, N_T], BF16, kind="ExternalInput")
    s_in = nc.dram_tensor("s_in", [SQ_N * STRIP_K, SQ_W], BF16,
                          kind="ExternalInput")
    # 4 full-tile tridiag blocks + identity + strip block-diag tri + strip I
    w_in = nc.dram_tensor("w_in", [P, 7 * P], BF16, kind="ExternalInput")
    stats_out = nc.dram_tensor("stats_out", [P, N_GROUPS], F32,
                               kind="ExternalOutput")

    KS = SQ_N * STRIP_K                  # 40 strip pack partitions

    with tile.TileContext(nc) as tc:
        with (
            tc.tile_pool(name="vpool", bufs=1) as vpool,
            tc.tile_pool(name="wpool", bufs=1) as wpool,
            tc.tile_pool(name="scratch", bufs=2) as spool,
            tc.tile_pool(name="psum", bufs=2, space="PSUM") as psum_pool,
        ):
            wall = wpool.tile([P, 7 * P], BF16)
            # strip stationary blocks ride a tiny first DMA so the strip's
            # matmuls start ~1.5us earlier than the full weights transfer.
            # (All on the Sync queue: moving these to the Scalar HWDGE queue
            # was tried and regressed ~4us — the issue cost lands on the Act
            # sequencer and the queues share the same 16 DMA engines.)
            nc.sync.dma_start(wall[0:KS, 5 * P:7 * P],
                              w_in[0:KS, 5 * P:7 * P])
            stats = wpool.tile([P, N_GROUPS], F32)

            # strip pack first, then whole full tiles, in compute order
            st = vpool.tile([KS, SQ_W], BF16)
            nc.sync.dma_start(st[:], s_in[:])
            nc.sync.dma_start(wall[:, 0:5 * P], w_in[:, 0:5 * P])
            vh = {}
            # tile 0 as two SEPARATE half-tiles (overlapping 12 cols): tile
            # dependency tracking is per-tile, so a single tile filled by two
            # DMAs makes every consumer wait for both halves (v8 trace); split
            # tiles let tile 0's group-0 work start ~3.5us earlier.
            B1 = 2040                    # vt0b covers global cols [2040,4096)
            vt0a = vpool.tile([P, 2052], BF16)
            nc.sync.dma_start(vt0a[:], v_in[0:P, 0:2052])
            vt0b = vpool.tile([P, N_T - B1], BF16)
            nc.sync.dma_start(vt0b[:], v_in[0:P, B1:N_T])
            for t in range(1, N_FULL):
                vt = vpool.tile([P, N_T], BF16, tag=f"v{t}")
                r0 = TILE_STARTS[t]
                nc.sync.dma_start(vt[:], v_in[r0:r0 + P, :])
                vh[t] = vt

            ident = wall[:, 4 * P:5 * P]
            s_tri = wall[0:KS, 5 * P:6 * P]
            s_ident = wall[0:KS, 6 * P:7 * P]

            # ---- strip (12-block pack): fills the pipeline ramp-up ----
            ws = spool.tile([KS, SQ_BW], BF16, tag="ws")
            nc.vector.tensor_tensor(out=ws[:], in0=st[0:KS, 2:SQ_W],
                                    in1=st[0:KS, 0:SQ_W - 2],
                                    op=mybir.AluOpType.subtract)
            s_ps = psum_pool.tile([P, 2048], F32, tag="ps")
            nc.tensor.matmul(s_ps[:, 0:SQ_BW], lhsT=s_tri,
                             rhs=st[0:KS, 1:1 + SQ_BW],
                             start=True, stop=False)
            nc.tensor.matmul(s_ps[:, 0:SQ_BW], lhsT=s_ident,
                             rhs=ws[0:KS, 0:SQ_BW],
                             start=False, stop=True)
            s_sq = spool.tile([P, 2048], F32, tag="sq")
            nc.scalar.activation(s_sq[:, 0:SQ_BW], s_ps[:, 0:SQ_BW],
                                 mybir.ActivationFunctionType.Square,
                                 accum_out=stats[:, U_STRIP:U_STRIP + 1])
            # ---- full tiles ----
            for t in range(N_FULL):
                tri = wall[:, t * P:(t + 1) * P]

                # t-shift sub in two group-sized pieces so group 0's identity
                # matmuls only wait on the first piece; tile 0 uses separate
                # half-tiles (and split w tiles) for the same reason
                if t == 0:
                    w0a = spool.tile([P, 2051], BF16)
                    nc.vector.tensor_tensor(
                        out=w0a[:, 1:2051], in0=vt0a[:, 2:2052],
                        in1=vt0a[:, 0:2050], op=mybir.AluOpType.subtract)
                    w0b = spool.tile([P, 2046], BF16)
                    nc.vector.tensor_tensor(
                        out=w0b[:], in0=vt0b[:, 10:2056],
                        in1=vt0b[:, 8:2054], op=mybir.AluOpType.subtract)

                    def vsl(g, off, cw):
                        c = G_BASE[g] + off
                        return (vt0a[:, c:c + cw] if g == 0
                                else vt0b[:, c - B1:c - B1 + cw])

                    def wsl(g, off, cw):
                        c = G_BASE[g] + off
                        return (w0a[:, c:c + cw] if g == 0
                                else w0b[:, c - G_BASE[1]:c - G_BASE[1] + cw])
                else:
                    vt = vh[t]
                    wt = spool.tile([P, N_T], BF16, tag=f"w{t}")
                    nc.vector.tensor_tensor(
                        out=wt[:, 1:2051], in0=vt[:, 2:2052],
                        in1=vt[:, 0:2050], op=mybir.AluOpType.subtract)
                    nc.vector.tensor_tensor(
                        out=wt[:, 2051:N_T - 1], in0=vt[:, 2052:N_T],
                        in1=vt[:, 2050:N_T - 2], op=mybir.AluOpType.subtract)

                    def vsl(g, off, cw, vt=vt):
                        c = G_BASE[g] + off
                        return vt[:, c:c + cw]

                    def wsl(g, off, cw, wt=wt):
                        c = G_BASE[g] + off
                        return wt[:, c:c + cw]

                ps0 = psum_pool.tile([P, 2048], F32, tag="ps")
                ps1 = psum_pool.tile([P, 2048], F32, tag="ps")
                ps = [ps0, ps1]

                def chunks(g):
                    for off in range(0, G_W[g], 512):
                        yield off, min(512, G_W[g] - off)

                # per group: tri then identity-add then square, so each PSUM
                # group drains as early as possible (no cross-group barrier)
                for g in (0, 1):
                    for off, cw in chunks(g):
                        nc.tensor.matmul(
                            ps[g][:, off:off + cw], lhsT=tri,
                            rhs=vsl(g, off, cw), start=True, stop=False)
                    for off, cw in chunks(g):
                        nc.tensor.matmul(
                            ps[g][:, off:off + cw], lhsT=ident,
                            rhs=wsl(g, off, cw), start=False, stop=True)
                    u = 2 * t + g
                    sq = spool.tile([P, 2048], F32, tag="sq")
                    if (t, g) in SQ_DVE:
                        rc = spool.tile([P, 2048], F32, tag="rc")
                        nc.vector.tensor_copy(rc[:, 0:G_W[g]],
                                              ps[g][:, 0:G_W[g]])
                        nc.vector.scalar_tensor_tensor(
                            out=sq[:, 0:G_W[g]], in0=rc[:, 0:G_W[g]],
                            scalar=1.0, in1=rc[:, 0:G_W[g]],
                            op0=mybir.AluOpType.mult,
                            op1=mybir.AluOpType.mult,
                            accum_out=stats[:, u:u + 1])
                    elif (t, g) == (3, 1):
                        # final group: square in halves so the first 1024
                        # cols (whose PSUM banks stop earlier) drain while
                        # PE finishes the last matmuls; 2nd half accumulates
                        # into the spare stats column
                        nc.scalar.activation(
                            sq[:, 0:1024], ps[g][:, 0:1024],
                            mybir.ActivationFunctionType.Square,
                            accum_out=stats[:, u:u + 1])
                        nc.scalar.activation(
                            sq[:, 1024:G_W[g]], ps[g][:, 1024:G_W[g]],
                            mybir.ActivationFunctionType.Square,
                            accum_out=stats[:, U_PROBE:U_PROBE + 1])
                    else:
                        nc.scalar.activation(
                            sq[:, 0:G_W[g]], ps[g][:, 0:G_W[g]],
                            mybir.ActivationFunctionType.Square,
                            accum_out=stats[:, u:u + 1])

            nc.scalar.dma_start(stats_out[:, 0:5], stats[:, 0:5])
            nc.sync.dma_start(stats_out[:, 5:], stats[:, 5:])

    nc.compile()
    return nc


def _host_inputs_and_masks(V: np.ndarray, S: np.ndarray):
    lo, mid, hi = _stencil_coeffs(S)
    c2 = float(C_T) ** 2
    KS = SQ_N * STRIP_K

    in_maps = []
    masks = []
    junk_subs = []

    for c in range(N_CORES):
        rows = np.clip(np.arange(512 * c - 1, 512 * c + 513), 0, N_S - 1)
        v_shard = np.ascontiguousarray(V[rows, :]).astype(BF16NP)
        strip16 = v_shard[STRIP_START:STRIP_START + STRIP_K, :]

        # strip pack: block q covers global out cols [342q, 342q+342)
        pack = np.zeros((KS, SQ_W), BF16NP)
        for q in range(SQ_N):
            cols = np.clip(np.arange(SQ_BW * q - 1, SQ_BW * q + SQ_W - 1),
                           0, N_T - 1)
            pack[STRIP_K * q:STRIP_K * (q + 1), :] = strip16[:, cols]

        w = np.zeros((P, 7 * P), np.float64)
        w[:, 4 * P:5 * P] = np.eye(P)
        mask = np.zeros((P, N_GROUPS), np.float32)

        for t in range(N_FULL):
            t0 = TILE_STARTS[t]
            for m in range(1, 127):
                L = t0 + m
                g = 512 * c - 1 + L
                if not (1 <= g <= N_S - 2):
                    continue
                w[m - 1, t * P + m] = lo[g]
                w[m, t * P + m] = mid[g]
                w[m + 1, t * P + m] = hi[g]
                mask[m, 2 * t:2 * t + 2] = c2
                if t == 3:
                    mask[m, U_PROBE] = c2

        # strip block-diagonal tri / identity stationaries (row stride 10)
        strip_rows_valid = []
        for m in range(1, 9):
            g = 512 * c - 1 + STRIP_START + m
            if not (1 <= g <= N_S - 2):
                continue
            strip_rows_valid.append((m, g))
            for q in range(SQ_N):
                w[STRIP_K * q + m - 1, 5 * P + STRIP_K * q + m] = lo[g]
                w[STRIP_K * q + m, 5 * P + STRIP_K * q + m] = mid[g]
                w[STRIP_K * q + m + 1, 5 * P + STRIP_K * q + m] = hi[g]
                mask[STRIP_K * q + m, U_STRIP] = c2
        for q in range(SQ_N):
            for k in range(STRIP_K):
                w[STRIP_K * q + k, 6 * P + STRIP_K * q + k] = 1.0

        # host-side subtraction of the junk columns summed into the strip
        # stats: global col 0 (block 0, m=0) and the clip-padded tail of
        # block 11 (out cols >= 4094's successor, m 333..341)
        p64 = pack.astype(np.float64)
        junk = 0.0
        junk_cols = [(0, 0)] + [(SQ_N - 1, m)
                                for m in range(N_T - 1 - SQ_BW * (SQ_N - 1),
                                               SQ_BW)]
        for q, m in junk_cols:
            b = STRIP_K * q
            for mr, g in strip_rows_valid:
                r = (lo[g] * p64[b + mr - 1, m + 1]
                     + mid[g] * p64[b + mr, m + 1]
                     + hi[g] * p64[b + mr + 1, m + 1]
                     + np.float64(BF16NP(p64[b + mr, m + 2]
                                         - p64[b + mr, m])))
                junk += c2 * r * r
        junk_subs.append(junk)

        in_maps.append({"v_in": v_shard, "s_in": pack,
                        "w_in": w.astype(np.float32).astype(BF16NP)})
        masks.append(mask)
    return in_maps, masks, junk_subs


_LAST_RESULTS = None  # stashed BassKernelResults (for the test harness)


def kernel(V_norm: np.ndarray, S_grid: np.ndarray, t_grid: np.ndarray):
    global _PROGRAM, _LAST_RESULTS

    V = np.asarray(V_norm, dtype=np.float32).reshape(N_S, N_T)
    S = np.asarray(S_grid, dtype=np.float32).reshape(N_S)
    t = np.asarray(t_grid, dtype=np.float32).reshape(N_T)

    if _PROGRAM is None:
        _PROGRAM = _build_program()
    nc = _PROGRAM

    in_maps, masks, junk_subs = _host_inputs_and_masks(V, S)
    trace = bool(os.environ.get("BSLOSS_TRACE"))
    res = run_bass_kernel_spmd(nc, in_maps, core_ids=list(range(N_CORES)),
                               trace=trace)
    _LAST_RESULTS = res

    pde_sum = 0.0
    for c in range(N_CORES):
        stats = res.results[c]["stats_out"].astype(np.float64)
        pde_sum += float((masks[c].astype(np.float64) * stats).sum())
        pde_sum -= junk_subs[c]
    n_int = (N_S - 2) * (N_T - 2)
    pde_loss = pde_sum / n_int

    # ---- boundary losses on host (tiny O(N) edge terms), float64 ----
    V64 = V.astype(np.float64)
    S64 = S.astype(np.float64)
    t64 = t.astype(np.float64)

    loss_S0 = float((V64[0, :] ** 2).sum() / N_T)

    tau = 1.0 - t64
    V_ff = 1.0 - K * np.exp(-R * tau) / SMAX
    loss_Smax = float(((V64[N_S - 1, :] - V_ff) ** 2).sum() / N_T)

    x = SOFTPLUS_BETA * (S64 - K / SMAX)
    payoff = (np.maximum(x, 0.0) + np.log1p(np.exp(-np.abs(x)))) / SOFTPLUS_BETA
    diff_T = V64[:, N_T - 1] - payoff
    abs_d = np.abs(diff_T)
    huber = np.where(abs_d < HUBER_DELTA, 0.5 * diff_T ** 2,
                     HUBER_DELTA * (abs_d - 0.5 * HUBER_DELTA))
    loss_T = float(huber.sum() / N_S)

    total = L_PDE * pde_loss + L_BC * loss_Smax + L_TC * loss_T
    return (np.float32(total), np.float32(pde_loss), np.float32(loss_S0),
            np.float32(loss_Smax), np.float32(loss_T))
